# revision 6
# baseline (speedup 1.0000x reference)
"""Trainium2 Bass kernel for nn_CharacterModel (char-LSTM + masked sigmoid attention).

Strategy:
  - Data-parallel over words: core c gets sorted words c::8 (lengths stay sorted
    descending per core), 1024 words/core.
  - Ragged packed-sequence processing: at step t only the first n_t words are
    active (n_t = ceil(#global words with len > t / 8), identical across cores).
  - Layout: hidden/gate dims on partitions, words on the free axis.
    h, c, res are [128, 2, 1024] tiles (hid = j*128 + p).
  - bf16 matmuls (fp32 is 4x slower on PE). Bias enters via a 65th "ones" row of
    the per-step input matmul. At t=0 h is a broadcast h_init, so W_hh@h0+b is
    folded into the t=0 bias row and the recurrent matmul is skipped.
  - h is double-buffered by step parity, so the attention pipeline for step t
    (which reads h_t) can lag into step t+1 without blocking the recurrence.
  - Attention runs once per step over the whole active range: the dot a.h is
    computed with broadcast-weight matmuls (lhsT[p, m] = a[p] for all m), so
    the [128, n] PSUM result holds the dot replicated on every partition --
    sigmoid of that IS the broadcast weight row, no ones-matmul needed. Words
    are sorted by length, so only the single ceil-rounding boundary column can
    be stale padding; it is zeroed with a 1-column DVE multiply against a
    per-core 0/1 mask.
  - res += w*h via DVE bf16 mult + GPSIMD fp32 accumulate (off critical path).
  - Host folds the unsort permutation + reshape into the unshard.
"""

import math
import os

import numpy as np
import ml_dtypes

N_WORDS = 8192
MAX_WLEN = 16
EMB = 64
HID = 256
N_SENT = 256
SENT_LEN = 32
NCORES = 8
W = N_WORDS // NCORES  # 1024 words per core
CHUNK = 512

BF16 = ml_dtypes.bfloat16

_BUILD_CACHE = {}
last_result = None  # stashes the most recent BassKernelResults (for profiling)


def _build(nts, nas):
    """Build + schedule the Bass program.

    nts: per-step matmul word counts (padded to >=256 so the PE HAM clock
         stays at 8/8). nas: true active word counts (ceil of global/8);
         every engine except the PE is clamped to this range.
    """
    import concourse.tile as tile
    import concourse.mybir as mybir
    from concourse import bacc

    f32 = mybir.dt.float32
    bf16 = mybir.dt.bfloat16
    AF = mybir.ActivationFunctionType
    OP = mybir.AluOpType

    nc = bacc.Bacc("TRN2", name="char_lstm")

    d_embs = nc.dram_tensor("embs", [MAX_WLEN, EMB + 1, W], bf16, kind="ExternalInput")
    d_wih0 = nc.dram_tensor("wih0", [EMB + 1, 4 * HID], bf16, kind="ExternalInput")
    d_wih = nc.dram_tensor("wih", [EMB + 1, 4 * HID], bf16, kind="ExternalInput")
    d_whh = nc.dram_tensor("whh", [2, 128, 4 * HID], bf16, kind="ExternalInput")
    d_attnb = nc.dram_tensor("attnb", [2, 128, 128], bf16, kind="ExternalInput")
    d_mask = nc.dram_tensor("maskc", [128, MAX_WLEN], bf16, kind="ExternalInput")
    d_c0 = nc.dram_tensor("c0", [128, 2], f32, kind="ExternalInput")
    d_out = nc.dram_tensor("res", [128, 2, W], f32, kind="ExternalOutput")

    # gate processing order: g first so the DVE i*g multiply can start after
    # two ACT calls instead of three. torch gate column order is i,f,g,o.
    GORDER = [2, 0, 1, 3]  # g, i, f, o
    GFUNC = {0: AF.Sigmoid, 1: AF.Sigmoid, 2: AF.Tanh, 3: AF.Sigmoid}

    with tile.TileContext(nc) as tc:
        with (
            tc.tile_pool(name="const", bufs=1) as cp,
            tc.tile_pool(name="embp", bufs=3) as ep,
            tc.tile_pool(name="gatep", bufs=4) as gp,
            tc.tile_pool(name="workp", bufs=6) as wp,
            tc.tile_pool(name="attnp", bufs=2) as ap_,
            tc.tile_pool(name="state", bufs=1) as sp,
            tc.tile_pool(name="pgate", bufs=3, space="PSUM") as pg,
            tc.tile_pool(name="pattn", bufs=1, space="PSUM") as pa,
        ):
            # --- constants (critical-path DMAs on sync; bulk on gpsimd queue) ---
            wih0 = cp.tile([EMB + 1, 4 * HID], bf16, tag="wih0")
            nc.sync.dma_start(wih0[:], d_wih0[:])
            wih = cp.tile([EMB + 1, 4 * HID], bf16, tag="wih")
            nc.gpsimd.dma_start(wih[:], d_wih[:])
            whh = cp.tile([128, 2, 4 * HID], bf16, tag="whh")
            for k in range(2):
                nc.gpsimd.dma_start(whh[:, k, :], d_whh[k, :, :])
            attnb = cp.tile([128, 2, 128], bf16, tag="attnb")
            for j in range(2):
                nc.gpsimd.dma_start(attnb[:, j, :], d_attnb[j, :, :])
            maskc = cp.tile([128, MAX_WLEN], bf16, tag="maskc")
            nc.gpsimd.dma_start(maskc[:], d_mask[:])
            c0t = cp.tile([128, 2], f32, tag="c0")
            nc.gpsimd.dma_start(c0t[:], d_c0[:])

            # --- state (h double-buffered by step parity) ---
            hbuf = [sp.tile([128, 2, W], bf16, tag=f"h{p}", name=f"h{p}") for p in range(2)]
            c = sp.tile([128, 2, W], bf16, tag="c")
            res = sp.tile([128, 2, W], f32, tag="res")
            nc.vector.memset(res[:], 0.0)

            for t in range(MAX_WLEN):
                n = nts[t]
                na = nas[t]
                if n == 0:
                    break
                h = hbuf[t % 2]         # written this step
                hprev = hbuf[(t + 1) % 2]  # read by the recurrence
                wih_t = wih0 if t == 0 else wih
                embt = ep.tile([EMB + 1, W], bf16, tag="embt")
                if t == 0:
                    # split across queues so the first chunk's matmuls start asap
                    nc.sync.dma_start(embt[:, :CHUNK], d_embs[t, :, :CHUNK])
                    nc.scalar.dma_start(embt[:, CHUNK:n], d_embs[t, :, CHUNK:n])
                else:
                    nc.sync.dma_start(embt[:, :n], d_embs[t, :, :n])

                # chunking: big steps in 512s; single-chunk steps split in two
                # halves so PE pipelines across the serial ACT/DVE chain.
                if n > CHUNK:
                    bounds = list(range(0, n, CHUNK)) + [n]
                elif n > 128:
                    half = (n // 2 + 63) // 64 * 64
                    bounds = [0, half, n]
                else:
                    bounds = [0, n]

                for ci in range(len(bounds) - 1):
                    w0, we = bounds[ci], bounds[ci + 1]
                    cw = we - w0
                    aw = min(we, na) - w0  # active width for non-PE engines
                    # --- gates (PE -> PSUM, then ACT -> SBUF bf16) ---
                    gsb = {}
                    for gi in GORDER:
                        ps = pg.tile([128, 2, CHUNK], f32, tag="gates")
                        for j in range(2):
                            col = gi * 256 + j * 128
                            if t > 0:
                                nc.tensor.matmul(
                                    ps[:, j, :cw], whh[:, 0, col:col + 128],
                                    hprev[:, 0, w0:we], start=True, stop=False)
                                nc.tensor.matmul(
                                    ps[:, j, :cw], whh[:, 1, col:col + 128],
                                    hprev[:, 1, w0:we], start=False, stop=False)
                                nc.tensor.matmul(
                                    ps[:, j, :cw], wih_t[:, col:col + 128],
                                    embt[:, w0:we], start=False, stop=True)
                            else:
                                nc.tensor.matmul(
                                    ps[:, j, :cw], wih_t[:, col:col + 128],
                                    embt[:, w0:we], start=True, stop=True)
                        if aw > 0:
                            g_sb = gp.tile([128, 2, CHUNK], bf16, tag=f"g{gi}")
                            nc.scalar.activation(g_sb[:, :, :aw], ps[:, :, :aw], GFUNC[gi])
                            gsb[gi] = g_sb
                    if aw <= 0:
                        continue
                    gi_, gf_, gg_, go_ = gsb[0], gsb[1], gsb[2], gsb[3]

                    # --- cell update (DVE) ---
                    wa = w0 + aw
                    ig = wp.tile([128, 2, CHUNK], bf16, tag="ig")
                    nc.vector.tensor_tensor(ig[:, :, :aw], gi_[:, :, :aw], gg_[:, :, :aw], OP.mult)
                    if t == 0:
                        for j in range(2):
                            nc.vector.scalar_tensor_tensor(
                                c[:, j, w0:wa], gf_[:, j, :aw], c0t[:, j:j + 1],
                                ig[:, j, :aw], OP.mult, OP.add)
                    else:
                        nc.vector.tensor_tensor(c[:, :, w0:wa], gf_[:, :, :aw], c[:, :, w0:wa], OP.mult)
                        nc.vector.tensor_tensor(c[:, :, w0:wa], c[:, :, w0:wa], ig[:, :, :aw], OP.add)
                    tnc = wp.tile([128, 2, CHUNK], bf16, tag="tanhc")
                    nc.scalar.activation(tnc[:, :, :aw], c[:, :, w0:wa], AF.Tanh)
                    nc.vector.tensor_tensor(h[:, :, w0:wa], go_[:, :, :aw], tnc[:, :, :aw], OP.mult)

                # --- attention, step-wide over [0, n) in 512-wide windows
                #     (off the critical path: reads this step's parity buffer;
                #     each matmul's PSUM dest stays within one bank) ---
                nw = (na + CHUNK - 1) // CHUNK
                dps = pa.tile([128, 2, CHUNK], f32, tag="dot")
                for k in range(nw):
                    a0 = k * CHUNK
                    ae = min(na, a0 + CHUNK)
                    kw = ae - a0
                    nc.tensor.matmul(dps[:, k, :kw], attnb[:, 0, :], h[:, 0, a0:ae],
                                     start=True, stop=False)
                    nc.tensor.matmul(dps[:, k, :kw], attnb[:, 1, :], h[:, 1, a0:ae],
                                     start=False, stop=True)
                # one sigmoid covering all windows (any tail garbage is unread)
                wb = ap_.tile([128, 2, CHUNK], bf16, tag="wb")
                nc.scalar.activation(wb[:, :nw, :], dps[:, :nw, :], AF.Sigmoid)
                # boundary column: on cores with floor-count words this col is
                # stale padding; multiply by per-core 0/1 mask.
                if t > 0:
                    bk, bo = (na - 1) // CHUNK, (na - 1) % CHUNK
                    nc.vector.tensor_tensor(
                        wb[:, bk, bo:bo + 1], wb[:, bk, bo:bo + 1],
                        maskc[:, t:t + 1], OP.mult)
                t2 = ap_.tile([128, 2, W], bf16, tag="t2")
                for k in range(nw):
                    a0 = k * CHUNK
                    ae = min(na, a0 + CHUNK)
                    kw = ae - a0
                    for j in range(2):
                        nc.vector.tensor_tensor(t2[:, j, a0:ae], h[:, j, a0:ae],
                                                wb[:, k, :kw], OP.mult)
                nc.gpsimd.tensor_tensor(res[:, :, :na], res[:, :, :na], t2[:, :, :na], OP.add)

                # words [n_{t+1}, n_t) retire after this step: stream them out
                n_next = nts[t + 1] if t + 1 < MAX_WLEN else 0
                if n_next < n:
                    nc.sync.dma_start(d_out[:, :, n_next:n], res[:, :, n_next:n])

            if nts[MAX_WLEN - 1] > 0:
                nc.sync.dma_start(d_out[:, :, :nts[MAX_WLEN - 1]], res[:, :, :nts[MAX_WLEN - 1]])

    nc.compile()
    return nc


def _get_nc(nts, nas):
    key = (tuple(nts), tuple(nas))
    if key not in _BUILD_CACHE:
        _BUILD_CACHE[key] = _build(*key)
    return _BUILD_CACHE[key]


def kernel(chars, wordlens, word_orig_idx, emb_table, W_ih, W_hh, b_ih, b_hh,
           attn_w, h_init, c_init):
    global last_result
    from concourse.bass_utils import run_bass_kernel_spmd

    chars = np.asarray(chars)
    wordlens = np.asarray(wordlens)
    word_orig_idx = np.asarray(word_orig_idx)
    emb_table = np.asarray(emb_table, dtype=np.float32)
    W_ih = np.asarray(W_ih, dtype=np.float32)
    W_hh = np.asarray(W_hh, dtype=np.float32)
    b_ih = np.asarray(b_ih, dtype=np.float32)
    b_hh = np.asarray(b_hh, dtype=np.float32)
    attn_w = np.asarray(attn_w, dtype=np.float32)
    h_init = np.asarray(h_init, dtype=np.float32)
    c_init = np.asarray(c_init, dtype=np.float32)

    # per-step active word counts (identical schedule on every core).
    # Matmul ranges (nts) are padded up to 256 words so the PE duty stays high
    # enough that HAM doesn't re-throttle the clock to 4/8; all other engines
    # are clamped to the true ceil counts (nas).
    nas = tuple(int(math.ceil(int((wordlens > t).sum()) / NCORES)) for t in range(MAX_WLEN))
    nts = tuple(max(v, 256) if v > 0 else 0 for v in nas)
    nc = _get_nc(nts, nas)

    bias = b_ih + b_hh
    bias0 = bias + W_hh @ h_init
    wihT = np.concatenate([W_ih.T, bias[None, :]], axis=0)
    wih0T = np.concatenate([W_ih.T, bias0[None, :]], axis=0)
    # broadcast attention weights: attnb[j, p, m] = attn_w[j*128 + p] for all m
    w2 = attn_w.reshape(2, 128)
    attnb = np.broadcast_to(w2[:, :, None], (2, 128, 128)).copy()
    shared = {
        "wih": wihT.astype(BF16),
        "wih0": wih0T.astype(BF16),
        "whh": W_hh.T.reshape(2, 128, 4 * HID).astype(BF16),
        "attnb": attnb.astype(BF16),
        "c0": c_init.reshape(2, 128).T.copy().astype(np.float32),
    }

    steps = np.arange(MAX_WLEN)
    in_maps = []
    for cid in range(NCORES):
        idx = np.arange(W) * NCORES + cid
        embs = emb_table[chars[idx]]            # [W, 16, 64]
        embsT = np.ones((MAX_WLEN, EMB + 1, W), np.float32)
        embsT[:, :EMB, :] = embs.transpose(1, 2, 0)
        lens = wordlens[idx]
        # per-core 0/1 mask for the ceil-rounding boundary column of each step:
        # 1.0 if this core's true count reaches nts[t] (col nts[t]-1 active)
        c_t = (lens[None, :] > steps[:, None]).sum(axis=1)  # [16]
        maskv = (c_t >= np.asarray(nas)).astype(np.float32)  # [16]
        maskcv = np.broadcast_to(maskv[None, :], (128, MAX_WLEN)).copy()
        in_maps.append({
            **shared,
            "embs": embsT.astype(BF16),
            "maskc": maskcv.astype(BF16),
        })

    last_result = run_bass_kernel_spmd(
        nc, in_maps, core_ids=list(range(NCORES)),
        trace=bool(int(os.environ.get("KERNEL_TRACE", "0"))),
    )

    res_sorted = np.zeros((N_WORDS, HID), np.float32)
    for cid in range(NCORES):
        rc = np.asarray(last_result.results[cid]["res"])  # [128, 2, W]
        res_sorted[np.arange(W) * NCORES + cid] = rc.transpose(2, 1, 0).reshape(W, HID)

    out = np.zeros_like(res_sorted)
    out[word_orig_idx] = res_sorted
    return out.reshape(N_SENT, SENT_LEN, HID)


# revision 7
# speedup vs baseline: 1.3408x; 1.3408x over previous
"""Trainium2 Bass kernel for nn_CharacterModel (char-LSTM + masked sigmoid attention).

Strategy:
  - Data-parallel over words: core c gets sorted words c::8 (lengths stay sorted
    descending per core), 1024 words/core.
  - Ragged packed-sequence processing: at step t only the first n_t words are
    active (n_t = ceil(#global words with len > t / 8), identical across cores).
  - Layout: hidden/gate dims on partitions, words on the free axis.
    h, c, res are [128, 2, 1024] tiles (hid = j*128 + p).
  - bf16 matmuls (fp32 is 4x slower on PE). Bias enters via a 65th "ones" row of
    the per-step input matmul. At t=0 h is a broadcast h_init, so W_hh@h0+b is
    folded into the t=0 bias row and the recurrent matmul is skipped.
  - h is double-buffered by step parity, so the attention pipeline for step t
    (which reads h_t) can lag into step t+1 without blocking the recurrence.
  - Attention runs once per step over the whole active range: the dot a.h is
    computed with broadcast-weight matmuls (lhsT[p, m] = a[p] for all m), so
    the [128, n] PSUM result holds the dot replicated on every partition --
    sigmoid of that IS the broadcast weight row, no ones-matmul needed. Words
    are sorted by length, so only the single ceil-rounding boundary column can
    be stale padding; it is zeroed with a 1-column DVE multiply against a
    per-core 0/1 mask.
  - res += w*h via DVE bf16 mult + GPSIMD fp32 accumulate (off critical path).
  - Host folds the unsort permutation + reshape into the unshard.
"""

import math
import os

import numpy as np
import ml_dtypes

N_WORDS = 8192
MAX_WLEN = 16
EMB = 64
HID = 256
N_SENT = 256
SENT_LEN = 32
NCORES = 8
W = N_WORDS // NCORES  # 1024 words per core
CHUNK = 512

BF16 = ml_dtypes.bfloat16

_BUILD_CACHE = {}
last_result = None  # stashes the most recent BassKernelResults (for profiling)


def _build(nts, nas):
    """Build + schedule the Bass program.

    nts: per-step matmul word counts (padded to >=256 so the PE HAM clock
         stays at 8/8). nas: true active word counts (ceil of global/8);
         every engine except the PE is clamped to this range.
    """
    import concourse.tile as tile
    import concourse.mybir as mybir
    from concourse import bacc

    f32 = mybir.dt.float32
    bf16 = mybir.dt.bfloat16
    AF = mybir.ActivationFunctionType
    OP = mybir.AluOpType

    nc = bacc.Bacc("TRN2", name="char_lstm")

    d_embs = nc.dram_tensor("embs", [MAX_WLEN, EMB + 1, W], bf16, kind="ExternalInput")
    d_wih0 = nc.dram_tensor("wih0", [EMB + 1, 4 * HID], bf16, kind="ExternalInput")
    d_wih = nc.dram_tensor("wih", [EMB + 1, 4 * HID], bf16, kind="ExternalInput")
    d_whh = nc.dram_tensor("whh", [2, 128, 4 * HID], bf16, kind="ExternalInput")
    d_attnb = nc.dram_tensor("attnb", [2, 128, 128], bf16, kind="ExternalInput")
    d_mask = nc.dram_tensor("maskc", [128, MAX_WLEN], bf16, kind="ExternalInput")
    d_c0 = nc.dram_tensor("c0", [128, 2], f32, kind="ExternalInput")
    d_out = nc.dram_tensor("res", [128, 2, W], f32, kind="ExternalOutput")

    # gate processing order: g first so the DVE i*g multiply can start after
    # two ACT calls instead of three. torch gate column order is i,f,g,o.
    GORDER = [2, 0, 1, 3]  # g, i, f, o
    GFUNC = {0: AF.Sigmoid, 1: AF.Sigmoid, 2: AF.Tanh, 3: AF.Sigmoid}

    with tile.TileContext(nc) as tc:
        with (
            tc.tile_pool(name="const", bufs=1) as cp,
            tc.tile_pool(name="embp", bufs=3) as ep,
            tc.tile_pool(name="gatep", bufs=4) as gp,
            tc.tile_pool(name="workp", bufs=6) as wp,
            tc.tile_pool(name="attnp", bufs=2) as ap_,
            tc.tile_pool(name="state", bufs=1) as sp,
            tc.tile_pool(name="pgate", bufs=3, space="PSUM") as pg,
            tc.tile_pool(name="pattn", bufs=1, space="PSUM") as pa,
        ):
            # --- constants (critical-path DMAs on sync; bulk on gpsimd queue) ---
            wih0 = cp.tile([EMB + 1, 4 * HID], bf16, tag="wih0")
            nc.sync.dma_start(wih0[:], d_wih0[:])
            wih = cp.tile([EMB + 1, 4 * HID], bf16, tag="wih")
            nc.gpsimd.dma_start(wih[:], d_wih[:])
            whh = cp.tile([128, 2, 4 * HID], bf16, tag="whh")
            for k in range(2):
                nc.gpsimd.dma_start(whh[:, k, :], d_whh[k, :, :])
            attnb = cp.tile([128, 2, 128], bf16, tag="attnb")
            for j in range(2):
                nc.gpsimd.dma_start(attnb[:, j, :], d_attnb[j, :, :])
            maskc = cp.tile([128, MAX_WLEN], bf16, tag="maskc")
            nc.gpsimd.dma_start(maskc[:], d_mask[:])
            c0t = cp.tile([128, 2], f32, tag="c0")
            nc.gpsimd.dma_start(c0t[:], d_c0[:])

            # --- state (h double-buffered by step parity) ---
            hbuf = [sp.tile([128, 2, W], bf16, tag=f"h{p}", name=f"h{p}") for p in range(2)]
            c = sp.tile([128, 2, W], bf16, tag="c")
            res = sp.tile([128, 2, W], f32, tag="res")
            nc.vector.memset(res[:], 0.0)

            pending_attn = None
            for t in range(MAX_WLEN):
                n = nts[t]
                na = nas[t]
                if n == 0:
                    break
                h = hbuf[t % 2]         # written this step
                hprev = hbuf[(t + 1) % 2]  # read by the recurrence
                wih_t = wih0 if t == 0 else wih
                embt = ep.tile([EMB + 1, W], bf16, tag="embt")
                if t == 0:
                    # split across queues so the first chunk's matmuls start asap
                    nc.sync.dma_start(embt[:, :CHUNK], d_embs[t, :, :CHUNK])
                    nc.scalar.dma_start(embt[:, CHUNK:n], d_embs[t, :, CHUNK:n])
                else:
                    nc.sync.dma_start(embt[:, :n], d_embs[t, :, :n])

                # chunking: big steps in 512s; single-chunk steps split in two
                # halves so PE pipelines across the serial ACT/DVE chain.
                if n > CHUNK:
                    bounds = list(range(0, n, CHUNK)) + [n]
                elif n > 128:
                    half = (n // 2 + 63) // 64 * 64
                    bounds = [0, half, n]
                else:
                    bounds = [0, n]

                for ci in range(len(bounds) - 1):
                    w0, we = bounds[ci], bounds[ci + 1]
                    cw = we - w0
                    aw = min(we, na) - w0  # active width for non-PE engines
                    # emit the previous step's attention AFTER this step's first
                    # chunk of gate matmuls: its dot matmuls then never stall
                    # the in-order PE queue (h of step t-1 is finished by now).
                    if ci == 1 and pending_attn is not None:
                        pending_attn()
                        pending_attn = None
                    # --- gates (PE -> PSUM, then ACT -> SBUF bf16) ---
                    gsb = {}
                    for gi in GORDER:
                        ps = pg.tile([128, 2, CHUNK], f32, tag="gates")
                        for j in range(2):
                            col = gi * 256 + j * 128
                            if t > 0:
                                nc.tensor.matmul(
                                    ps[:, j, :cw], whh[:, 0, col:col + 128],
                                    hprev[:, 0, w0:we], start=True, stop=False)
                                nc.tensor.matmul(
                                    ps[:, j, :cw], whh[:, 1, col:col + 128],
                                    hprev[:, 1, w0:we], start=False, stop=False)
                                nc.tensor.matmul(
                                    ps[:, j, :cw], wih_t[:, col:col + 128],
                                    embt[:, w0:we], start=False, stop=True)
                            else:
                                nc.tensor.matmul(
                                    ps[:, j, :cw], wih_t[:, col:col + 128],
                                    embt[:, w0:we], start=True, stop=True)
                        if aw > 0:
                            g_sb = gp.tile([128, 2, CHUNK], bf16, tag=f"g{gi}")
                            nc.scalar.activation(g_sb[:, :, :aw], ps[:, :, :aw], GFUNC[gi])
                            gsb[gi] = g_sb
                    if aw <= 0:
                        continue
                    gi_, gf_, gg_, go_ = gsb[0], gsb[1], gsb[2], gsb[3]

                    # --- cell update (DVE) ---
                    wa = w0 + aw
                    ig = wp.tile([128, 2, CHUNK], bf16, tag="ig")
                    nc.vector.tensor_tensor(ig[:, :, :aw], gi_[:, :, :aw], gg_[:, :, :aw], OP.mult)
                    if t == 0:
                        for j in range(2):
                            nc.vector.scalar_tensor_tensor(
                                c[:, j, w0:wa], gf_[:, j, :aw], c0t[:, j:j + 1],
                                ig[:, j, :aw], OP.mult, OP.add)
                    else:
                        nc.vector.tensor_tensor(c[:, :, w0:wa], gf_[:, :, :aw], c[:, :, w0:wa], OP.mult)
                        nc.vector.tensor_tensor(c[:, :, w0:wa], c[:, :, w0:wa], ig[:, :, :aw], OP.add)
                    tnc = wp.tile([128, 2, CHUNK], bf16, tag="tanhc")
                    nc.scalar.activation(tnc[:, :, :aw], c[:, :, w0:wa], AF.Tanh)
                    nc.vector.tensor_tensor(h[:, :, w0:wa], go_[:, :, :aw], tnc[:, :, :aw], OP.mult)

                # --- attention, step-wide over [0, na) in 512-wide windows.
                #     Deferred: emitted mid-way through the NEXT step's gate
                #     matmuls (h is parity-buffered, so it stays valid). ---
                def make_attn(t, h, n, na):
                    def emit():
                        nw = (na + CHUNK - 1) // CHUNK
                        dps = pa.tile([128, 2, CHUNK], f32, tag="dot", name="dps")
                        for k in range(nw):
                            a0 = k * CHUNK
                            ae = min(na, a0 + CHUNK)
                            kw = ae - a0
                            nc.tensor.matmul(dps[:, k, :kw], attnb[:, 0, :],
                                             h[:, 0, a0:ae], start=True, stop=False)
                            nc.tensor.matmul(dps[:, k, :kw], attnb[:, 1, :],
                                             h[:, 1, a0:ae], start=False, stop=True)
                        # one sigmoid covering all windows (tail garbage unread)
                        wb = ap_.tile([128, 2, CHUNK], bf16, tag="wb", name="wb")
                        nc.scalar.activation(wb[:, :nw, :], dps[:, :nw, :], AF.Sigmoid)
                        # boundary column: on cores with floor-count words this
                        # col is stale padding; multiply by per-core 0/1 mask.
                        if t > 0:
                            bk, bo = (na - 1) // CHUNK, (na - 1) % CHUNK
                            nc.vector.tensor_tensor(
                                wb[:, bk, bo:bo + 1], wb[:, bk, bo:bo + 1],
                                maskc[:, t:t + 1], OP.mult)
                        t2 = ap_.tile([128, 2, W], bf16, tag="t2", name="t2")
                        for k in range(nw):
                            a0 = k * CHUNK
                            ae = min(na, a0 + CHUNK)
                            kw = ae - a0
                            for j in range(2):
                                nc.vector.tensor_tensor(t2[:, j, a0:ae], h[:, j, a0:ae],
                                                        wb[:, k, :kw], OP.mult)
                        nc.gpsimd.tensor_tensor(res[:, :, :na], res[:, :, :na],
                                                t2[:, :, :na], OP.add)
                        # words [n_{t+1}, n_t) retired after step t: stream out
                        n_next = nts[t + 1] if t + 1 < MAX_WLEN else 0
                        if n_next < n:
                            nc.sync.dma_start(d_out[:, :, n_next:n], res[:, :, n_next:n])
                    return emit

                if pending_attn is not None:  # single-chunk step: not yet emitted
                    pending_attn()
                pending_attn = make_attn(t, h, n, na)

            if pending_attn is not None:
                pending_attn()
            if nts[MAX_WLEN - 1] > 0:
                nc.sync.dma_start(d_out[:, :, :nts[MAX_WLEN - 1]], res[:, :, :nts[MAX_WLEN - 1]])

    nc.compile()
    return nc


def _get_nc(nts, nas):
    key = (tuple(nts), tuple(nas))
    if key not in _BUILD_CACHE:
        _BUILD_CACHE[key] = _build(*key)
    return _BUILD_CACHE[key]


def kernel(chars, wordlens, word_orig_idx, emb_table, W_ih, W_hh, b_ih, b_hh,
           attn_w, h_init, c_init):
    global last_result
    from concourse.bass_utils import run_bass_kernel_spmd

    chars = np.asarray(chars)
    wordlens = np.asarray(wordlens)
    word_orig_idx = np.asarray(word_orig_idx)
    emb_table = np.asarray(emb_table, dtype=np.float32)
    W_ih = np.asarray(W_ih, dtype=np.float32)
    W_hh = np.asarray(W_hh, dtype=np.float32)
    b_ih = np.asarray(b_ih, dtype=np.float32)
    b_hh = np.asarray(b_hh, dtype=np.float32)
    attn_w = np.asarray(attn_w, dtype=np.float32)
    h_init = np.asarray(h_init, dtype=np.float32)
    c_init = np.asarray(c_init, dtype=np.float32)

    # per-step active word counts (identical schedule on every core).
    # Matmul ranges (nts) are padded up to 256 words so the PE duty stays high
    # enough that HAM doesn't re-throttle the clock to 4/8; all other engines
    # are clamped to the true ceil counts (nas).
    nas = tuple(int(math.ceil(int((wordlens > t).sum()) / NCORES)) for t in range(MAX_WLEN))
    nts = tuple(max(v, 256) if v > 0 else 0 for v in nas)
    nc = _get_nc(nts, nas)

    bias = b_ih + b_hh
    bias0 = bias + W_hh @ h_init
    wihT = np.concatenate([W_ih.T, bias[None, :]], axis=0)
    wih0T = np.concatenate([W_ih.T, bias0[None, :]], axis=0)
    # broadcast attention weights: attnb[j, p, m] = attn_w[j*128 + p] for all m
    w2 = attn_w.reshape(2, 128)
    attnb = np.broadcast_to(w2[:, :, None], (2, 128, 128)).copy()
    shared = {
        "wih": wihT.astype(BF16),
        "wih0": wih0T.astype(BF16),
        "whh": W_hh.T.reshape(2, 128, 4 * HID).astype(BF16),
        "attnb": attnb.astype(BF16),
        "c0": c_init.reshape(2, 128).T.copy().astype(np.float32),
    }

    steps = np.arange(MAX_WLEN)
    in_maps = []
    for cid in range(NCORES):
        idx = np.arange(W) * NCORES + cid
        embs = emb_table[chars[idx]]            # [W, 16, 64]
        embsT = np.ones((MAX_WLEN, EMB + 1, W), np.float32)
        embsT[:, :EMB, :] = embs.transpose(1, 2, 0)
        lens = wordlens[idx]
        # per-core 0/1 mask for the ceil-rounding boundary column of each step:
        # 1.0 if this core's true count reaches nts[t] (col nts[t]-1 active)
        c_t = (lens[None, :] > steps[:, None]).sum(axis=1)  # [16]
        maskv = (c_t >= np.asarray(nas)).astype(np.float32)  # [16]
        maskcv = np.broadcast_to(maskv[None, :], (128, MAX_WLEN)).copy()
        in_maps.append({
            **shared,
            "embs": embsT.astype(BF16),
            "maskc": maskcv.astype(BF16),
        })

    last_result = run_bass_kernel_spmd(
        nc, in_maps, core_ids=list(range(NCORES)),
        trace=bool(int(os.environ.get("KERNEL_TRACE", "0"))),
    )

    res_sorted = np.zeros((N_WORDS, HID), np.float32)
    for cid in range(NCORES):
        rc = np.asarray(last_result.results[cid]["res"])  # [128, 2, W]
        res_sorted[np.arange(W) * NCORES + cid] = rc.transpose(2, 1, 0).reshape(W, HID)

    out = np.zeros_like(res_sorted)
    out[word_orig_idx] = res_sorted
    return out.reshape(N_SENT, SENT_LEN, HID)
